# revision 3
# baseline (speedup 1.0000x reference)
"""GCN 2-layer classifier on 8 TRN2 NeuronCores.

Strategy (dst-sharded graph parallel, gather/scatter via GPSIMD + scan):
  - Nodes sharded 8 ways by id range (NSH=12544 logical rows per core, core 7
    zero-padded).  Each core receives ONLY its own inputs, packed into a
    single uint8 blob per core (one host->device transfer): x shard in bf16,
    int16 edge-index streams, f32 scalars/weights.
  - Each core computes hs1 = (x_shard @ W1) * dinv, transposes it to
    [16, NSH], and AllGathers across cores; the concatenated result IS the
    feature-major table layout table_T[(bank, feat), node_in_bank] =
    [128, NSH], DMA'd once into SBUF.  Same for layer 2.
  - Edges sorted by dst on the host, bucketed per (core, src-bank,
    dst-range-chunk) into uniform-length int16 index streams (SPMD-identical
    structure, data differs per core).  Per chunk:
      * GPSIMD ap_gather pulls hs[src] along the free axis for all 8 banks in
        parallel (each Q7 core serves its bank's 16 feature partitions).
      * DVE tensor_tensor_scan computes a plain prefix sum over the
        dst-sorted message stream.
      * a second ap_gather extracts the prefix at per-dst segment boundaries;
        adjacent differences give per-(bank,dst) partial sums.
      * one PE matmul per 128 dsts contracts the partition axis against a
        block-identity selector, summing the 8 banks AND transposing to
        [dst, feat] in PSUM.
  - Symmetric normalization folds into the tables: out = dinv*(agg+hs[d]) + b
    with hs = h*dinv, so there is no per-edge norm work.
  - Layer 2 aggregates 16-dim features first (A@h commutes with @W2), then
    applies W2 + b2 and log-softmax on-chip.
"""

import sys

import numpy as np

sys.path.insert(0, "/opt/trn_rl_repo")

N_NODES = 100000
N_EDGES = 3200000
D_IN, D_HID, D_OUT = 128, 16, 2
NCORES = 8
P = 128
NSH = 12544          # shard rows per core (98 * 128)
TILES = NSH // P     # 98
NCHUNK = 14          # dst-range chunks per core
DCH = NSH // NCHUNK  # 896 dsts per chunk (= 7 node tiles)
TPC = DCH // P       # 7 tiles per chunk
NBANK = 8


def _host_prep(edge_index):
    """Sort edges by dst, bucket per (core, src-bank, dst-chunk), build
    uniform int16 gather/extraction index streams."""
    src = np.ascontiguousarray(edge_index[0]).astype(np.int64)
    dst = np.ascontiguousarray(edge_index[1]).astype(np.int64)

    deg = np.bincount(dst, minlength=N_NODES).astype(np.float64) + 1.0
    dinv = (1.0 / np.sqrt(deg)).astype(np.float32)

    order = np.argsort(dst, kind="stable")
    src_s = src[order]
    dst_s = dst[order]
    bank_s = src_s // NSH

    # cell id = ((core * NBANK) + bank) * NCHUNK + chunk, edges within a cell
    # stay dst-sorted under a stable sort by cell
    core_s = dst_s // NSH
    chunk_s = (dst_s % NSH) // DCH
    cell = (core_s * NBANK + bank_s) * NCHUNK + chunk_s
    cell_order = np.argsort(cell, kind="stable")
    src_c = src_s[cell_order]
    dst_c = dst_s[cell_order]
    cell_c = cell[cell_order]

    ncells = NCORES * NBANK * NCHUNK
    counts = np.bincount(cell_c, minlength=ncells)
    starts = np.zeros(ncells + 1, dtype=np.int64)
    np.cumsum(counts, out=starts[1:])

    # uniform padded stream length: slot 0 is a zero sentinel
    # round to multiples of 32 so every per-chunk int16 index slice starts
    # 4-byte aligned (GPSIMD reads indices in 32-bit words)
    nidx = int(counts.max()) + 1
    nidx = ((nidx + 31) // 32) * 32
    nx = DCH + 1
    nx = ((nx + 31) // 32) * 32

    gidx = np.zeros((NCORES, P, NCHUNK * (nidx // 16)), dtype=np.int16)
    xidx = np.zeros((NCORES, P, NCHUNK * (nx // 16)), dtype=np.int16)

    src_local = (src_c % NSH).astype(np.int32)
    rel_dst = (dst_c % NSH) % DCH

    for c in range(NCORES):
        for b in range(NBANK):
            rows = slice(b * 16, (b + 1) * 16)
            for k in range(NCHUNK):
                g = (c * NBANK + b) * NCHUNK + k
                a, e = starts[g], starts[g + 1]
                n = e - a
                # gather stream: [0] + bank-local src ids + pads(0)
                stream = np.zeros(nidx, dtype=np.int16)
                stream[1:1 + n] = src_local[a:e]
                gidx[c, rows, k * (nidx // 16):(k + 1) * (nidx // 16)] = (
                    stream.reshape(nidx // 16, 16).T
                )
                # extraction stream: prefix positions [0, cum(0), ..,
                # cum(DCH-1)] then pads repeating the last position
                cum = np.zeros(nx, dtype=np.int16)
                cnt = np.bincount(rel_dst[a:e], minlength=DCH)
                cum[1:DCH + 1] = np.cumsum(cnt)
                cum[DCH + 1:] = cum[DCH]
                xidx[c, rows, k * (nx // 16):(k + 1) * (nx // 16)] = (
                    cum.reshape(nx // 16, 16).T
                )

    return gidx, xidx, dinv, nidx, nx


def _blob_layout(nidx, nx):
    """Byte offsets of each logical tensor inside the packed per-core blob."""
    GC = NCHUNK * (nidx // 16)
    XC = NCHUNK * (nx // 16)
    off = {}
    nb = 0
    off["x"] = nb
    nb += NSH * D_IN * 2          # bf16
    off["gidx"] = nb
    nb += P * GC * 2              # int16
    off["xidx"] = nb
    nb += P * XC * 2              # int16
    assert nb % 4 == 0
    off["dinv"] = nb
    nb += NSH * 4
    off["W1"] = nb
    nb += D_IN * D_HID * 4
    off["b1"] = nb
    nb += D_HID * 4
    off["W2"] = nb
    nb += D_HID * D_OUT * 4
    off["b2"] = nb
    nb += D_OUT * 4
    off["selmat"] = nb
    nb += P * D_HID * 4
    nb = (nb + 511) // 512 * 512
    return off, nb, GC, XC


def _build_program(nidx, nx):
    from contextlib import ExitStack

    import concourse.bass as bass
    import concourse.tile as tile
    from concourse import bacc, mybir
    from concourse.masks import make_identity

    f32 = mybir.dt.float32
    bf16 = mybir.dt.bfloat16
    i16 = mybir.dt.int16
    u8 = mybir.dt.uint8

    off, nb, GC, XC = _blob_layout(nidx, nx)

    nc = bacc.Bacc(
        "TRN2",
        target_bir_lowering=False,
        debug=False,
        enable_asserts=False,
        num_devices=NCORES,
    )

    # ---- kernel I/O: one packed input blob, one output ----
    blob = nc.dram_tensor("blob", [nb], u8, kind="ExternalInput")
    out_d = nc.dram_tensor("out", [NSH, D_OUT], f32, kind="ExternalOutput")

    xv = blob.bitcast(bf16)
    iv = blob.bitcast(i16)
    fv = blob.bitcast(f32)

    # internal DRAM: transposed shard bounces + transposed (gathered) tables
    ag_in1 = nc.dram_tensor("ag_in1", [D_HID, NSH], f32)
    ag_in2 = nc.dram_tensor("ag_in2", [D_HID, NSH], f32)
    table1 = nc.dram_tensor("table1", [P, NSH], f32, addr_space="Shared")
    table2 = nc.dram_tensor("table2", [P, NSH], f32, addr_space="Shared")

    groups = [list(range(NCORES))]

    with tile.TileContext(nc) as tc, ExitStack() as ctx:
        singles = ctx.enter_context(tc.tile_pool(name="singles", bufs=1))
        xpool = ctx.enter_context(tc.tile_pool(name="xload", bufs=3))
        xtp = ctx.enter_context(tc.tile_pool(name="xtsb", bufs=3))
        msgp = ctx.enter_context(tc.tile_pool(name="msg", bufs=2))
        scnp = ctx.enter_context(tc.tile_pool(name="scn", bufs=2))
        extp = ctx.enter_context(tc.tile_pool(name="ext", bufs=2))
        psA = ctx.enter_context(tc.tile_pool(name="psA", bufs=2, space="PSUM"))
        psB = ctx.enter_context(tc.tile_pool(name="psB", bufs=2, space="PSUM"))
        psW = ctx.enter_context(tc.tile_pool(name="psW", bufs=3, space="PSUM"))

        # ---- constants (all APs are views into the packed blob) ----
        w1s = singles.tile([D_IN, D_HID], f32)
        nc.sync.dma_start(
            out=w1s[:],
            in_=bass.AP(fv, off["W1"] // 4, [[D_HID, D_IN], [1, D_HID]]))
        w2s = singles.tile([D_HID, D_OUT], f32)
        nc.sync.dma_start(
            out=w2s[:],
            in_=bass.AP(fv, off["W2"] // 4, [[D_OUT, D_HID], [1, D_OUT]]))
        b1s = singles.tile([P, D_HID], f32)
        nc.sync.dma_start(
            out=b1s[:],
            in_=bass.AP(fv, off["b1"] // 4, [[1, D_HID]])
            .unsqueeze(0).to_broadcast([P, D_HID]))
        b2s = singles.tile([P, D_OUT], f32)
        nc.sync.dma_start(
            out=b2s[:],
            in_=bass.AP(fv, off["b2"] // 4, [[1, D_OUT]])
            .unsqueeze(0).to_broadcast([P, D_OUT]))
        sels = singles.tile([P, D_HID], f32)
        nc.sync.dma_start(
            out=sels[:],
            in_=bass.AP(fv, off["selmat"] // 4, [[D_HID, P], [1, D_HID]]))
        dinvs = singles.tile([P, TILES], f32)
        nc.sync.dma_start(
            out=dinvs[:], in_=bass.AP(fv, off["dinv"] // 4, [[1, P], [P, TILES]]))
        ident = singles.tile([P, P], f32)
        make_identity(nc, ident[:])

        gidx = singles.tile([P, GC], i16)
        nc.sync.dma_start(out=gidx[:], in_=bass.AP(iv, off["gidx"] // 2,
                                                   [[GC, P], [1, GC]]))
        xidx = singles.tile([P, XC], i16)
        nc.sync.dma_start(out=xidx[:], in_=bass.AP(iv, off["xidx"] // 2,
                                                   [[XC, P], [1, XC]]))

        hs1_loc = singles.tile([P, TILES, D_HID], f32)
        hs2_loc = singles.tile([P, TILES, D_HID], f32)
        agg1 = singles.tile([P, TILES, D_HID], f32)
        agg2 = singles.tile([P, TILES, D_HID], f32)
        tableT = singles.tile([P, NSH], f32)

        dinv_bc = dinvs[:].unsqueeze(2).to_broadcast([P, TILES, D_HID])

        # ---- shard -> feature-major full-graph table via AllGather ----
        def shard_to_table(hs_loc, ag_in, table):
            GRP = 7
            for g in range(TILES // GRP):
                stg = xtp.tile([D_HID, GRP * P], f32, tag="stg")
                for j in range(GRP):
                    t = g * GRP + j
                    tp = psA.tile([D_HID, P], f32, space="PSUM", tag="shT")
                    nc.tensor.transpose(tp[:], hs_loc[:, t, :], ident[:])
                    nc.vector.tensor_copy(stg[:, j * P:(j + 1) * P], tp[:])
                nc.sync.dma_start(
                    out=bass.AP(ag_in, g * GRP * P, [[NSH, D_HID], [1, GRP * P]]),
                    in_=stg[:],
                )
            nc.gpsimd.collective_compute(
                "AllGather", mybir.AluOpType.bypass, replica_groups=groups,
                ins=[ag_in.ap().opt()], outs=[table.ap().opt()],
            )
            nc.sync.dma_start(out=tableT[:], in_=table[:, :])

        # ---- phase A: hs1 = (x_shard @ W1) * dinv (node-major) ----
        for t in range(TILES):
            xt = xpool.tile([P, D_IN], bf16)
            nc.sync.dma_start(
                out=xt[:],
                in_=bass.AP(xv, off["x"] // 2 + t * P * D_IN,
                            [[D_IN, P], [1, D_IN]]))
            xtf = xpool.tile([P, D_IN], f32, tag="xf32")
            nc.vector.tensor_copy(xtf[:], xt[:])
            xt_ps = psA.tile([P, P], f32, space="PSUM", tag="shT")
            nc.tensor.transpose(xt_ps[:], xtf[:], ident[:])
            xT = xtp.tile([P, P], f32)
            nc.vector.tensor_copy(xT[:], xt_ps[:])
            h_ps = psB.tile([P, D_HID], f32, space="PSUM", tag="small")
            nc.tensor.matmul(out=h_ps[:], lhsT=xT[:], rhs=w1s[:], start=True, stop=True)
            nc.vector.tensor_scalar_mul(hs1_loc[:, t, :], h_ps[:], dinvs[:, t:t + 1])

        shard_to_table(hs1_loc, ag_in1, table1)

        # ---- edge aggregation ----
        def aggregate(aggbuf):
            for k in range(NCHUNK):
                msg = msgp.tile([P, nidx], f32, tag="msg")
                nc.gpsimd.ap_gather(
                    out_ap=msg[:], in_ap=tableT[:],
                    idxs_ap=gidx[:, k * (nidx // 16):(k + 1) * (nidx // 16)],
                    channels=P, num_elems=NSH, d=1, num_idxs=nidx,
                )
                nc.vector.memset(msg[:, 0:1], 0.0)
                scn = scnp.tile([P, nidx], f32, tag="scn")
                nc.vector.tensor_tensor_scan(
                    out=scn[:], data0=msg[:], data1=msg[:], initial=0.0,
                    op0=mybir.AluOpType.add, op1=mybir.AluOpType.bypass,
                )
                ex = extp.tile([P, nx], f32, tag="ex")
                nc.gpsimd.ap_gather(
                    out_ap=ex[:], in_ap=scn[:],
                    idxs_ap=xidx[:, k * (nx // 16):(k + 1) * (nx // 16)],
                    channels=P, num_elems=nidx, d=1, num_idxs=nx,
                )
                dif = extp.tile([P, DCH], f32, tag="dif")
                nc.vector.tensor_sub(dif[:], ex[:, 1:DCH + 1], ex[:, 0:DCH])
                for j in range(TPC):
                    ps = psW.tile([P, D_HID], f32, space="PSUM")
                    nc.tensor.matmul(
                        out=ps[:], lhsT=dif[:, j * P:(j + 1) * P], rhs=sels[:],
                        start=True, stop=True,
                    )
                    nc.vector.tensor_copy(aggbuf[:, k * TPC + j, :], ps[:])

        aggregate(agg1)

        # ---- layer-1 epilogue ----
        t1 = singles.tile([P, TILES, D_HID], f32)
        nc.vector.tensor_add(out=t1[:], in0=agg1[:], in1=hs1_loc[:])
        nc.vector.tensor_mul(out=t1[:], in0=t1[:], in1=dinv_bc)
        b1_bc = b1s[:].unsqueeze(1).to_broadcast([P, TILES, D_HID])
        nc.vector.tensor_add(out=t1[:], in0=t1[:], in1=b1_bc)
        nc.scalar.activation(out=t1[:], in_=t1[:], func=mybir.ActivationFunctionType.Relu)
        nc.vector.tensor_mul(out=hs2_loc[:], in0=t1[:], in1=dinv_bc)

        shard_to_table(hs2_loc, ag_in2, table2)

        aggregate(agg2)

        # ---- layer-2 epilogue: y = (dinv*(agg2+hs2)) @ W2 + b2; log_softmax
        t2 = singles.tile([P, TILES, D_HID], f32)
        nc.vector.tensor_add(out=t2[:], in0=agg2[:], in1=hs2_loc[:])
        nc.vector.tensor_mul(out=t2[:], in0=t2[:], in1=dinv_bc)

        fin = singles.tile([P, TILES, D_OUT], f32)
        for t in range(TILES):
            tp_ps = psA.tile([D_HID, P], f32, space="PSUM", tag="shT")
            nc.tensor.transpose(tp_ps[:], t2[:, t, :], ident[:])
            t2T = xtp.tile([D_HID, P], f32, tag="t2T")
            nc.vector.tensor_copy(t2T[:], tp_ps[:])
            y_ps = psB.tile([P, D_OUT], f32, space="PSUM", tag="small")
            nc.tensor.matmul(out=y_ps[:], lhsT=t2T[:], rhs=w2s[:], start=True, stop=True)
            nc.vector.tensor_add(out=fin[:, t, :], in0=y_ps[:], in1=b2s[:])

        # log-softmax over 2 classes, batched over [P, TILES]
        mx = singles.tile([P, TILES], f32)
        nc.vector.tensor_max(out=mx[:], in0=fin[:, :, 0], in1=fin[:, :, 1])
        mx_bc = mx[:].unsqueeze(2).to_broadcast([P, TILES, D_OUT])
        zc = singles.tile([P, TILES, D_OUT], f32)
        nc.vector.tensor_sub(out=zc[:], in0=fin[:], in1=mx_bc)
        ez = singles.tile([P, TILES, D_OUT], f32)
        nc.scalar.activation(out=ez[:], in_=zc[:], func=mybir.ActivationFunctionType.Exp)
        sm = singles.tile([P, TILES], f32)
        nc.vector.tensor_add(out=sm[:], in0=ez[:, :, 0], in1=ez[:, :, 1])
        ls = singles.tile([P, TILES], f32)
        nc.scalar.activation(out=ls[:], in_=sm[:], func=mybir.ActivationFunctionType.Ln)
        ls_bc = ls[:].unsqueeze(2).to_broadcast([P, TILES, D_OUT])
        res = singles.tile([P, TILES, D_OUT], f32)
        nc.vector.tensor_sub(out=res[:], in0=zc[:], in1=ls_bc)

        out_ap = bass.AP(out_d, 0, [[D_OUT, P], [P * D_OUT, TILES], [1, D_OUT]])
        nc.sync.dma_start(out=out_ap, in_=res[:])

    nc.compile()
    return nc


def _build_noop():
    """Tiny program for calibrating the PJRT/axon transport overhead."""
    from contextlib import ExitStack

    import concourse.tile as tile
    from concourse import bacc, mybir

    f32 = mybir.dt.float32
    nc = bacc.Bacc(
        "TRN2", target_bir_lowering=False, debug=False,
        enable_asserts=False, num_devices=NCORES,
    )
    z_in = nc.dram_tensor("z_in", [P, P], f32, kind="ExternalInput")
    z_out = nc.dram_tensor("z_out", [P, P], f32, kind="ExternalOutput")
    with tile.TileContext(nc) as tc, ExitStack() as ctx:
        sb = ctx.enter_context(tc.tile_pool(name="sb", bufs=1))
        t = sb.tile([P, P], f32)
        nc.sync.dma_start(out=t[:], in_=z_in[:, :])
        nc.sync.dma_start(out=z_out[:, :], in_=t[:])
    nc.compile()
    return nc


_CACHE = {}


def _make_in_maps(inputs_np, gidx, xidx, dinv):
    import ml_dtypes

    nidx = gidx.shape[2] // NCHUNK * 16 // 1
    # recover nidx/nx from array shapes
    nidx = gidx.shape[2] // NCHUNK * 16
    nx = xidx.shape[2] // NCHUNK * 16
    off, nb, GC, XC = _blob_layout(nidx, nx)

    x = np.asarray(inputs_np["x"], dtype=np.float32)
    x_pad = np.zeros((NCORES * NSH, D_IN), dtype=np.float32)
    x_pad[:N_NODES] = x
    x_bf = x_pad.astype(ml_dtypes.bfloat16)
    dinv_pad = np.ones(NCORES * NSH, dtype=np.float32)
    dinv_pad[:N_NODES] = dinv
    selmat = np.tile(np.eye(D_HID, dtype=np.float32), (NBANK, 1))

    w1 = np.ascontiguousarray(np.asarray(inputs_np["W1"], dtype=np.float32))
    b1 = np.ascontiguousarray(np.asarray(inputs_np["b1"], dtype=np.float32))
    w2 = np.ascontiguousarray(np.asarray(inputs_np["W2"], dtype=np.float32))
    b2 = np.ascontiguousarray(np.asarray(inputs_np["b2"], dtype=np.float32))

    in_maps = []
    for c in range(NCORES):
        blob = np.zeros(nb, np.uint8)

        def put(o, arr):
            raw = np.ascontiguousarray(arr).view(np.uint8).reshape(-1)
            blob[o:o + raw.size] = raw

        put(off["x"], x_bf[c * NSH:(c + 1) * NSH])
        put(off["gidx"], gidx[c])
        put(off["xidx"], xidx[c])
        put(off["dinv"], dinv_pad[c * NSH:(c + 1) * NSH])
        put(off["W1"], w1)
        put(off["b1"], b1)
        put(off["W2"], w2)
        put(off["b2"], b2)
        put(off["selmat"], selmat)
        in_maps.append({"blob": blob})
    return in_maps


def kernel(x, W1, b1, W2, b2, edge_index):
    from concourse.bass_utils import run_bass_kernel_spmd

    inputs_np = {"x": x, "W1": W1, "b1": b1, "W2": W2, "b2": b2}
    edge_index = np.asarray(edge_index)

    gidx, xidx, dinv, nidx, nx = _host_prep(edge_index)

    key = (nidx, nx)
    if key not in _CACHE:
        _CACHE[key] = _build_program(nidx, nx)
    nc = _CACHE[key]

    in_maps = _make_in_maps(inputs_np, gidx, xidx, dinv)

    res = run_bass_kernel_spmd(nc, in_maps, core_ids=list(range(NCORES)))
    shards = [res.results[c]["out"] for c in range(NCORES)]
    out = np.concatenate(shards, axis=0)[:N_NODES]
    return np.ascontiguousarray(out.astype(np.float32))


# revision 5
# speedup vs baseline: 1.0590x; 1.0590x over previous
"""GCN 2-layer classifier on 8 TRN2 NeuronCores.

Strategy (dst-sharded graph parallel, feature-major end-to-end):
  - Nodes sharded 8 ways by id range (NSH=12544 logical rows per core, core 7
    zero-padded).  Each core receives ONLY its own inputs, packed into a
    single uint8 blob per core (one host->device transfer): (x*dinv)^T shard
    in bf16 (pre-scaled + pre-transposed on the host), int16 edge-index
    streams, bf16/f32 weights.
  - All node features on device live feature-major [16, NSH]: hs1 = W1^T @
    xsT is 28 PE matmuls straight out of SBUF (no transposes anywhere).
    The per-layer full-graph table is built by AllGathering the bf16
    [16, NSH] shard: the concatenation IS the table layout
    table_T[(bank, feat), node_in_bank] = [128, NSH], upconverted once to
    f32 in SBUF for the gather.
  - Edges sorted by dst on the host, bucketed per (core, src-bank,
    dst-range-chunk) into uniform-length int16 index streams (SPMD-identical
    structure, data differs per core).  Per chunk:
      * GPSIMD ap_gather pulls hs[src] along the free axis for all 8 banks in
        parallel (each Q7 core serves its bank's 16 feature partitions).
      * DVE tensor_tensor_scan computes a plain prefix sum over the
        dst-sorted message stream.
      * a second ap_gather extracts the prefix at per-dst segment boundaries;
        adjacent differences give per-(bank,dst) partial sums.
      * one PE matmul per 448 dsts contracts the partition axis against a
        block-identity selector, summing the 8 banks into feature-major
        [16, 448] partial sums, accumulated directly onto the hs shard
        (self-loop term pre-seeded).
  - Symmetric normalization folds into the tables: out = dinv*(agg+hs[d]) + b
    with hs = h*dinv, so there is no per-edge norm work.
  - Layer 2 aggregates 16-dim features first (A@h commutes with @W2); the
    final 2-class log-softmax uses the closed form out0 = -softplus(y1-y0),
    out1 = (y1-y0) + out0, needing only the single projected difference
    d = (W2[:,1]-W2[:,0])^T @ t2 + (b2[1]-b2[0]).
"""

import sys

import numpy as np

sys.path.insert(0, "/opt/trn_rl_repo")

N_NODES = 100000
N_EDGES = 3200000
D_IN, D_HID, D_OUT = 128, 16, 2
NCORES = 8
P = 128
NSH = 12544          # shard rows per core (98 * 128)
NCHUNK = 14          # dst-range chunks per core
DCH = NSH // NCHUNK  # 896 dsts per chunk
BLK = 448            # matmul free-dim block (2 per chunk, 28 per shard)
NBLK = NSH // BLK    # 28
NBANK = 8


def _host_prep(edge_index):
    """Sort edges by dst, bucket per (core, src-bank, dst-chunk), build
    uniform int16 gather/extraction index streams."""
    src = np.ascontiguousarray(edge_index[0]).astype(np.int64)
    dst = np.ascontiguousarray(edge_index[1]).astype(np.int64)

    deg = np.bincount(dst, minlength=N_NODES).astype(np.float64) + 1.0
    dinv = (1.0 / np.sqrt(deg)).astype(np.float32)

    order = np.argsort(dst, kind="stable")
    src_s = src[order]
    dst_s = dst[order]
    bank_s = src_s // NSH

    # cell id = ((core * NBANK) + bank) * NCHUNK + chunk, edges within a cell
    # stay dst-sorted under a stable sort by cell
    core_s = dst_s // NSH
    chunk_s = (dst_s % NSH) // DCH
    cell = (core_s * NBANK + bank_s) * NCHUNK + chunk_s
    cell_order = np.argsort(cell, kind="stable")
    src_c = src_s[cell_order]
    dst_c = dst_s[cell_order]
    cell_c = cell[cell_order]

    ncells = NCORES * NBANK * NCHUNK
    counts = np.bincount(cell_c, minlength=ncells)
    starts = np.zeros(ncells + 1, dtype=np.int64)
    np.cumsum(counts, out=starts[1:])

    # uniform padded stream length: slot 0 is a zero sentinel
    # round to multiples of 32 so every per-chunk int16 index slice starts
    # 4-byte aligned (GPSIMD reads indices in 32-bit words)
    nidx = int(counts.max()) + 1
    nidx = ((nidx + 31) // 32) * 32
    nx = DCH + 1
    nx = ((nx + 31) // 32) * 32

    gidx = np.zeros((NCORES, P, NCHUNK * (nidx // 16)), dtype=np.int16)
    xidx = np.zeros((NCORES, P, NCHUNK * (nx // 16)), dtype=np.int16)

    src_local = (src_c % NSH).astype(np.int32)
    rel_dst = (dst_c % NSH) % DCH

    for c in range(NCORES):
        for b in range(NBANK):
            rows = slice(b * 16, (b + 1) * 16)
            for k in range(NCHUNK):
                g = (c * NBANK + b) * NCHUNK + k
                a, e = starts[g], starts[g + 1]
                n = e - a
                # gather stream: [0] + bank-local src ids + pads(0)
                stream = np.zeros(nidx, dtype=np.int16)
                stream[1:1 + n] = src_local[a:e]
                gidx[c, rows, k * (nidx // 16):(k + 1) * (nidx // 16)] = (
                    stream.reshape(nidx // 16, 16).T
                )
                # extraction stream: prefix positions [0, cum(0), ..,
                # cum(DCH-1)] then pads repeating the last position
                cum = np.zeros(nx, dtype=np.int16)
                cnt = np.bincount(rel_dst[a:e], minlength=DCH)
                cum[1:DCH + 1] = np.cumsum(cnt)
                cum[DCH + 1:] = cum[DCH]
                xidx[c, rows, k * (nx // 16):(k + 1) * (nx // 16)] = (
                    cum.reshape(nx // 16, 16).T
                )

    return gidx, xidx, dinv, nidx, nx


def _blob_layout(nidx, nx):
    """Byte offsets of each logical tensor inside the packed per-core blob."""
    GC = NCHUNK * (nidx // 16)
    XC = NCHUNK * (nx // 16)
    off = {}
    nb = 0
    off["xsT"] = nb
    nb += D_IN * NSH * 2          # bf16, pre-scaled + transposed x shard
    off["gidx"] = nb
    nb += P * GC * 2              # int16
    off["xidx"] = nb
    nb += P * XC * 2              # int16
    off["dinv"] = nb
    nb += NSH * 2                 # bf16
    off["W1"] = nb
    nb += D_IN * D_HID * 2        # bf16
    off["wd"] = nb
    nb += D_HID * 2               # bf16, W2[:,1]-W2[:,0]
    assert nb % 4 == 0
    off["selmat"] = nb
    nb += P * D_HID * 4           # f32
    off["b1"] = nb
    nb += D_HID * 4               # f32
    off["bd"] = nb
    nb += 4                       # f32, b2[1]-b2[0]
    nb = (nb + 511) // 512 * 512
    return off, nb, GC, XC


def _build_program(nidx, nx):
    from contextlib import ExitStack

    import concourse.bass as bass
    import concourse.tile as tile
    from concourse import bacc, mybir

    f32 = mybir.dt.float32
    bf16 = mybir.dt.bfloat16
    i16 = mybir.dt.int16
    u8 = mybir.dt.uint8

    off, nb, GC, XC = _blob_layout(nidx, nx)

    nc = bacc.Bacc(
        "TRN2",
        target_bir_lowering=False,
        debug=False,
        enable_asserts=False,
        num_devices=NCORES,
    )

    # ---- kernel I/O: one packed input blob, one (transposed) output ----
    blob = nc.dram_tensor("blob", [nb], u8, kind="ExternalInput")
    out_d = nc.dram_tensor("out", [D_OUT, NSH], f32, kind="ExternalOutput")

    bv = blob.bitcast(bf16)
    iv = blob.bitcast(i16)
    fv = blob.bitcast(f32)

    # internal DRAM: shard bounce + gathered tables (bf16 on the wire)
    ag_in1 = nc.dram_tensor("ag_in1", [D_HID, NSH], bf16)
    ag_in2 = nc.dram_tensor("ag_in2", [D_HID, NSH], bf16)
    table1 = nc.dram_tensor("table1", [P, NSH], bf16, addr_space="Shared")
    table2 = nc.dram_tensor("table2", [P, NSH], bf16, addr_space="Shared")

    groups = [list(range(NCORES))]

    with tile.TileContext(nc) as tc, ExitStack() as ctx:
        singles = ctx.enter_context(tc.tile_pool(name="singles", bufs=1))
        stream = ctx.enter_context(tc.tile_pool(name="stream", bufs=2))
        extp = ctx.enter_context(tc.tile_pool(name="ext", bufs=2))
        smalls = ctx.enter_context(tc.tile_pool(name="smalls", bufs=1))
        psA = ctx.enter_context(tc.tile_pool(name="psA", bufs=2, space="PSUM"))
        psD = ctx.enter_context(tc.tile_pool(name="psD", bufs=2, space="PSUM"))

        # ---- constants (all APs are views into the packed blob) ----
        w1bf = singles.tile([D_IN, D_HID], bf16)
        nc.sync.dma_start(
            out=w1bf[:],
            in_=bass.AP(bv, off["W1"] // 2, [[D_HID, D_IN], [1, D_HID]]))
        wdbf = singles.tile([D_HID, 1], bf16)
        nc.sync.dma_start(
            out=wdbf[:], in_=bass.AP(bv, off["wd"] // 2, [[1, D_HID], [1, 1]]))
        sels = singles.tile([P, D_HID], f32)
        nc.sync.dma_start(
            out=sels[:],
            in_=bass.AP(fv, off["selmat"] // 4, [[D_HID, P], [1, D_HID]]))
        b1col = singles.tile([D_HID, 1], f32)
        nc.sync.dma_start(
            out=b1col[:], in_=bass.AP(fv, off["b1"] // 4, [[1, D_HID], [1, 1]]))
        bdsc = singles.tile([1, 1], f32)
        nc.sync.dma_start(
            out=bdsc[:], in_=bass.AP(fv, off["bd"] // 4, [[1, 1], [1, 1]]))
        dinvT = singles.tile([D_HID, NSH], bf16)
        nc.sync.dma_start(
            out=dinvT[:],
            in_=bass.AP(bv, off["dinv"] // 2, [[1, NSH]])
            .unsqueeze(0).to_broadcast([D_HID, NSH]))

        gidx = singles.tile([P, GC], i16)
        nc.sync.dma_start(out=gidx[:], in_=bass.AP(iv, off["gidx"] // 2,
                                                   [[GC, P], [1, GC]]))
        xidx = singles.tile([P, XC], i16)
        nc.sync.dma_start(out=xidx[:], in_=bass.AP(iv, off["xidx"] // 2,
                                                   [[XC, P], [1, XC]]))

        hs1T = singles.tile([D_HID, NSH], bf16)
        hs2T = singles.tile([D_HID, NSH], bf16)
        tableT = singles.tile([P, NSH], f32)

        # ---- phase A: hs1^T = W1^T @ (dinv * x)^T, straight from SBUF ----
        xsT = stream.tile([P, NSH], bf16, tag="big", name="xsT")
        nc.sync.dma_start(
            out=xsT[:], in_=bass.AP(bv, off["xsT"] // 2, [[NSH, P], [1, NSH]]))
        for blk in range(NBLK):
            cols = slice(blk * BLK, (blk + 1) * BLK)
            ps = psA.tile([D_HID, BLK], f32, space="PSUM", tag="agg")
            nc.tensor.matmul(out=ps[:], lhsT=w1bf[:], rhs=xsT[:, cols],
                             start=True, stop=True)
            nc.vector.tensor_copy(hs1T[:, cols], ps[:])

        # ---- shard -> feature-major full-graph table via bf16 AllGather ----
        def make_table(hsT, ag_in, table, tname):
            nc.sync.dma_start(out=ag_in.ap(), in_=hsT[:])
            nc.gpsimd.collective_compute(
                "AllGather", mybir.AluOpType.bypass, replica_groups=groups,
                ins=[ag_in.ap().opt()], outs=[table.ap().opt()],
            )
            tmp = stream.tile([P, NSH], bf16, tag="big", name=tname)
            nc.sync.dma_start(out=tmp[:], in_=table[:, :])
            nc.vector.tensor_copy(tableT[:], tmp[:])

        # ---- edge aggregation: hsT[:, d] += sum_banks(segment sums) ----
        def aggregate(hsT):
            for k in range(NCHUNK):
                msg = stream.tile([P, nidx], f32, tag="big", name=f"msg{k}")
                nc.gpsimd.ap_gather(
                    out_ap=msg[:], in_ap=tableT[:],
                    idxs_ap=gidx[:, k * (nidx // 16):(k + 1) * (nidx // 16)],
                    channels=P, num_elems=NSH, d=1, num_idxs=nidx,
                )
                nc.vector.memset(msg[:, 0:1], 0.0)
                scn = stream.tile([P, nidx], f32, tag="big", name=f"scn{k}")
                nc.vector.tensor_tensor_scan(
                    out=scn[:], data0=msg[:], data1=msg[:], initial=0.0,
                    op0=mybir.AluOpType.add, op1=mybir.AluOpType.bypass,
                )
                ex = extp.tile([P, nx], f32, tag="ex")
                nc.gpsimd.ap_gather(
                    out_ap=ex[:], in_ap=scn[:],
                    idxs_ap=xidx[:, k * (nx // 16):(k + 1) * (nx // 16)],
                    channels=P, num_elems=nidx, d=1, num_idxs=nx,
                )
                dif = extp.tile([P, DCH], f32, tag="dif")
                nc.vector.tensor_sub(dif[:], ex[:, 1:DCH + 1], ex[:, 0:DCH])
                for j in range(2):
                    ps = psA.tile([D_HID, BLK], f32, space="PSUM", tag="agg")
                    nc.tensor.matmul(
                        out=ps[:], lhsT=sels[:], rhs=dif[:, j * BLK:(j + 1) * BLK],
                        start=True, stop=True,
                    )
                    cols = slice(k * DCH + j * BLK, k * DCH + (j + 1) * BLK)
                    nc.vector.tensor_add(out=hsT[:, cols], in0=ps[:],
                                         in1=hsT[:, cols])

        make_table(hs1T, ag_in1, table1, "tb1")
        aggregate(hs1T)

        # ---- layer-1 epilogue (feature-major, in place) ----
        nc.vector.tensor_mul(out=hs1T[:], in0=hs1T[:], in1=dinvT[:])
        nc.scalar.activation(out=hs1T[:], in_=hs1T[:],
                             func=mybir.ActivationFunctionType.Relu,
                             bias=b1col[:])
        nc.vector.tensor_mul(out=hs2T[:], in0=hs1T[:], in1=dinvT[:])

        make_table(hs2T, ag_in2, table2, "tb2")
        aggregate(hs2T)

        # ---- layer-2 epilogue + closed-form 2-class log-softmax ----
        # t2 = dinv*(agg2+hs2); d = wd^T t2 + bd;
        # out0 = -softplus(d); out1 = d + out0
        nc.vector.tensor_mul(out=hs2T[:], in0=hs2T[:], in1=dinvT[:])

        for blk in range(NBLK):
            cols = slice(blk * BLK, (blk + 1) * BLK)
            ps = psD.tile([1, BLK], f32, space="PSUM", tag="d")
            nc.tensor.matmul(out=ps[:], lhsT=wdbf[:], rhs=hs2T[:, cols],
                             start=True, stop=True)
            ex_d = smalls.tile([1, BLK], f32, tag="e")
            nc.scalar.activation(out=ex_d[:], in_=ps[:],
                                 func=mybir.ActivationFunctionType.Exp,
                                 bias=bdsc[:])
            sp = smalls.tile([1, BLK], f32, tag="sp")
            nc.scalar.activation(out=sp[:], in_=ex_d[:],
                                 func=mybir.ActivationFunctionType.Ln,
                                 bias=1.0)
            o0 = smalls.tile([1, BLK], f32, tag="o0")
            nc.vector.tensor_scalar_mul(o0[:], sp[:], -1.0)
            dsb = smalls.tile([1, BLK], f32, tag="dsb")
            nc.scalar.activation(out=dsb[:], in_=ps[:],
                                 func=mybir.ActivationFunctionType.Identity,
                                 bias=bdsc[:])
            o1 = smalls.tile([1, BLK], f32, tag="o1")
            nc.vector.tensor_sub(o1[:], dsb[:], sp[:])
            nc.sync.dma_start(
                out=bass.AP(out_d, blk * BLK, [[NSH, 1], [1, BLK]]),
                in_=o0[:])
            nc.sync.dma_start(
                out=bass.AP(out_d, NSH + blk * BLK, [[NSH, 1], [1, BLK]]),
                in_=o1[:])

    nc.compile()
    return nc


def _build_noop():
    """Tiny program for calibrating the PJRT/axon transport overhead."""
    from contextlib import ExitStack

    import concourse.tile as tile
    from concourse import bacc, mybir

    f32 = mybir.dt.float32
    nc = bacc.Bacc(
        "TRN2", target_bir_lowering=False, debug=False,
        enable_asserts=False, num_devices=NCORES,
    )
    z_in = nc.dram_tensor("z_in", [P, P], f32, kind="ExternalInput")
    z_out = nc.dram_tensor("z_out", [P, P], f32, kind="ExternalOutput")
    with tile.TileContext(nc) as tc, ExitStack() as ctx:
        sb = ctx.enter_context(tc.tile_pool(name="sb", bufs=1))
        t = sb.tile([P, P], f32)
        nc.sync.dma_start(out=t[:], in_=z_in[:, :])
        nc.sync.dma_start(out=z_out[:, :], in_=t[:])
    nc.compile()
    return nc


_CACHE = {}


def _make_in_maps(inputs_np, gidx, xidx, dinv):
    import ml_dtypes

    nidx = gidx.shape[2] // NCHUNK * 16
    nx = xidx.shape[2] // NCHUNK * 16
    off, nb, GC, XC = _blob_layout(nidx, nx)

    x = np.asarray(inputs_np["x"], dtype=np.float32)
    dinv_pad = np.ones(NCORES * NSH, dtype=np.float32)
    dinv_pad[:N_NODES] = dinv
    xs_pad = np.zeros((NCORES * NSH, D_IN), dtype=np.float32)
    xs_pad[:N_NODES] = x * dinv[:, None]
    dinv_bf = dinv_pad.astype(ml_dtypes.bfloat16)
    selmat = np.tile(np.eye(D_HID, dtype=np.float32), (NBANK, 1))

    w1_bf = np.ascontiguousarray(
        np.asarray(inputs_np["W1"], dtype=np.float32)).astype(ml_dtypes.bfloat16)
    b1 = np.ascontiguousarray(np.asarray(inputs_np["b1"], dtype=np.float32))
    w2 = np.asarray(inputs_np["W2"], dtype=np.float32)
    b2 = np.asarray(inputs_np["b2"], dtype=np.float32)
    wd_bf = np.ascontiguousarray(w2[:, 1] - w2[:, 0]).astype(ml_dtypes.bfloat16)
    bd = np.array([b2[1] - b2[0]], dtype=np.float32)

    in_maps = []
    for c in range(NCORES):
        blob = np.zeros(nb, np.uint8)

        def put(o, arr):
            raw = np.ascontiguousarray(arr).view(np.uint8).reshape(-1)
            blob[o:o + raw.size] = raw

        xsT_c = np.ascontiguousarray(
            xs_pad[c * NSH:(c + 1) * NSH].T).astype(ml_dtypes.bfloat16)
        put(off["xsT"], xsT_c)
        put(off["gidx"], gidx[c])
        put(off["xidx"], xidx[c])
        put(off["dinv"], dinv_bf[c * NSH:(c + 1) * NSH])
        put(off["W1"], w1_bf)
        put(off["wd"], wd_bf)
        put(off["selmat"], selmat)
        put(off["b1"], b1)
        put(off["bd"], bd)
        in_maps.append({"blob": blob})
    return in_maps


def kernel(x, W1, b1, W2, b2, edge_index):
    from concourse.bass_utils import run_bass_kernel_spmd

    inputs_np = {"x": x, "W1": W1, "b1": b1, "W2": W2, "b2": b2}
    edge_index = np.asarray(edge_index)

    gidx, xidx, dinv, nidx, nx = _host_prep(edge_index)

    key = (nidx, nx)
    if key not in _CACHE:
        _CACHE[key] = _build_program(nidx, nx)
    nc = _CACHE[key]

    in_maps = _make_in_maps(inputs_np, gidx, xidx, dinv)

    res = run_bass_kernel_spmd(nc, in_maps, core_ids=list(range(NCORES)))
    shards = [res.results[c]["out"] for c in range(NCORES)]  # each [2, NSH]
    out = np.concatenate(shards, axis=1).T[:N_NODES]
    return np.ascontiguousarray(out.astype(np.float32))


# revision 9
# speedup vs baseline: 1.2513x; 1.1815x over previous
"""GCN 2-layer classifier on 8 TRN2 NeuronCores.

Strategy (dst-sharded graph parallel, feature-major end-to-end):
  - Nodes sharded 8 ways by id range (NSH=12544 logical rows per core, core 7
    zero-padded).  Each core receives ONLY its own inputs, packed into a
    single uint8 blob per core (one host->device transfer): (x*dinv)^T shard
    in bf16 (pre-scaled + pre-transposed on the host), int16 edge-index
    streams, bf16/f32 weights.
  - All node features on device live feature-major [16, NSH]: hs1 = W1^T @
    xsT is 28 PE matmuls straight out of SBUF (no transposes anywhere).
    The per-layer full-graph table is built by AllGathering the bf16
    [16, NSH] shard: the concatenation IS the table layout
    table_T[(bank, feat), node_in_bank] = [128, NSH], upconverted once to
    f32 in SBUF for the gather.
  - Edges sorted by dst on the host, bucketed per (core, src-bank,
    dst-range-chunk) into uniform-length int16 index streams (SPMD-identical
    structure, data differs per core).  Per chunk:
      * GPSIMD ap_gather pulls hs[src] along the free axis for all 8 banks in
        parallel (each Q7 core serves its bank's 16 feature partitions).
      * DVE tensor_tensor_scan computes a plain prefix sum over the
        dst-sorted message stream.
      * a second ap_gather extracts the prefix at per-dst segment boundaries;
        adjacent differences give per-(bank,dst) partial sums.
      * one PE matmul per 448 dsts contracts the partition axis against a
        block-identity selector, summing the 8 banks into feature-major
        [16, 448] partial sums, accumulated directly onto the hs shard
        (self-loop term pre-seeded).
  - Symmetric normalization folds into the tables: out = dinv*(agg+hs[d]) + b
    with hs = h*dinv, so there is no per-edge norm work.
  - Layer 2 aggregates 16-dim features first (A@h commutes with @W2); the
    final 2-class log-softmax uses the closed form out0 = -softplus(y1-y0),
    out1 = (y1-y0) + out0, needing only the single projected difference
    d = (W2[:,1]-W2[:,0])^T @ t2 + (b2[1]-b2[0]).
"""

import sys

import numpy as np

sys.path.insert(0, "/opt/trn_rl_repo")

N_NODES = 100000
N_EDGES = 3200000
D_IN, D_HID, D_OUT = 128, 16, 2
NCORES = 8
P = 128
NSH = 12544          # shard rows per core (98 * 128)
NCHUNK = 14          # dst-range chunks per core
DCH = NSH // NCHUNK  # 896 dsts per chunk
BLK = 448            # matmul free-dim block (2 per chunk, 28 per shard)
NBLK = NSH // BLK    # 28
NBANK = 8


def _host_prep(edge_index):
    """Sort edges by dst, bucket per (core, src-bank, dst-chunk), build
    uniform int16 gather/extraction index streams."""
    src = np.ascontiguousarray(edge_index[0]).astype(np.int64)
    dst = np.ascontiguousarray(edge_index[1]).astype(np.int64)

    deg = np.bincount(dst, minlength=N_NODES).astype(np.float64) + 1.0
    dinv = (1.0 / np.sqrt(deg)).astype(np.float32)

    order = np.argsort(dst, kind="stable")
    src_s = src[order]
    dst_s = dst[order]
    bank_s = src_s // NSH

    # cell id = ((core * NBANK) + bank) * NCHUNK + chunk, edges within a cell
    # stay dst-sorted under a stable sort by cell
    core_s = dst_s // NSH
    chunk_s = (dst_s % NSH) // DCH
    cell = (core_s * NBANK + bank_s) * NCHUNK + chunk_s
    cell_order = np.argsort(cell, kind="stable")
    src_c = src_s[cell_order]
    dst_c = dst_s[cell_order]
    cell_c = cell[cell_order]

    ncells = NCORES * NBANK * NCHUNK
    counts = np.bincount(cell_c, minlength=ncells)
    starts = np.zeros(ncells + 1, dtype=np.int64)
    np.cumsum(counts, out=starts[1:])

    # uniform padded stream length: slot 0 is a zero sentinel
    # round to multiples of 32 so every per-chunk int16 index slice starts
    # 4-byte aligned (GPSIMD reads indices in 32-bit words)
    nidx = int(counts.max()) + 1
    nidx = ((nidx + 31) // 32) * 32
    nx = DCH + 1
    nx = ((nx + 31) // 32) * 32

    gidx = np.zeros((NCORES, P, NCHUNK * (nidx // 16)), dtype=np.int16)
    xidx = np.zeros((NCORES, P, NCHUNK * (nx // 16)), dtype=np.int16)

    src_local = (src_c % NSH).astype(np.int32)
    rel_dst = (dst_c % NSH) % DCH

    for c in range(NCORES):
        for b in range(NBANK):
            rows = slice(b * 16, (b + 1) * 16)
            for k in range(NCHUNK):
                g = (c * NBANK + b) * NCHUNK + k
                a, e = starts[g], starts[g + 1]
                n = e - a
                # gather stream: [0] + bank-local src ids + pads(0)
                stream = np.zeros(nidx, dtype=np.int16)
                stream[1:1 + n] = src_local[a:e]
                gidx[c, rows, k * (nidx // 16):(k + 1) * (nidx // 16)] = (
                    stream.reshape(nidx // 16, 16).T
                )
                # extraction stream: prefix positions [0, cum(0), ..,
                # cum(DCH-1)] then pads repeating the last position
                cum = np.zeros(nx, dtype=np.int16)
                cnt = np.bincount(rel_dst[a:e], minlength=DCH)
                cum[1:DCH + 1] = np.cumsum(cnt)
                cum[DCH + 1:] = cum[DCH]
                xidx[c, rows, k * (nx // 16):(k + 1) * (nx // 16)] = (
                    cum.reshape(nx // 16, 16).T
                )

    return gidx, xidx, dinv, nidx, nx


def _blob_layout(nidx, nx):
    """Byte offsets of each logical tensor inside the packed per-core blob."""
    GC = NCHUNK * (nidx // 16)
    XC = NCHUNK * (nx // 16)
    off = {}
    nb = 0
    off["xsT"] = nb
    nb += D_IN * NSH * 2          # bf16, pre-scaled + transposed x shard
    off["gidx"] = nb
    nb += P * GC * 2              # int16
    off["xidx"] = nb
    nb += P * XC * 2              # int16
    off["dinv"] = nb
    nb += NSH * 2                 # bf16
    off["W1"] = nb
    nb += D_IN * D_HID * 2        # bf16
    off["wd"] = nb
    nb += D_HID * 2               # bf16, W2[:,1]-W2[:,0]
    assert nb % 4 == 0
    off["selmat"] = nb
    nb += P * D_HID * 4           # f32
    off["b1"] = nb
    nb += D_HID * 4               # f32
    off["bd"] = nb
    nb += 4                       # f32, b2[1]-b2[0]
    nb = (nb + 511) // 512 * 512
    return off, nb, GC, XC


def _build_program(nidx, nx):
    from contextlib import ExitStack

    import concourse.bass as bass
    import concourse.tile as tile
    from concourse import bacc, mybir

    f32 = mybir.dt.float32
    bf16 = mybir.dt.bfloat16
    i16 = mybir.dt.int16
    u8 = mybir.dt.uint8

    off, nb, GC, XC = _blob_layout(nidx, nx)

    nc = bacc.Bacc(
        "TRN2",
        target_bir_lowering=False,
        debug=False,
        enable_asserts=False,
        num_devices=NCORES,
    )

    # ---- kernel I/O: one packed input blob, one (transposed) bf16 output ----
    blob = nc.dram_tensor("blob", [nb], u8, kind="ExternalInput")
    out_d = nc.dram_tensor("out", [D_OUT, NSH], bf16, kind="ExternalOutput")

    bv = blob.bitcast(bf16)
    iv = blob.bitcast(i16)
    fv = blob.bitcast(f32)

    # internal DRAM: shard bounce + gathered tables (bf16 on the wire)
    ag_in1 = nc.dram_tensor("ag_in1", [D_HID, NSH], bf16)
    ag_in2 = nc.dram_tensor("ag_in2", [D_HID, NSH], bf16)
    table1 = nc.dram_tensor("table1", [P, NSH], bf16, addr_space="Shared")
    table2 = nc.dram_tensor("table2", [P, NSH], bf16, addr_space="Shared")

    groups = [list(range(NCORES))]

    with tile.TileContext(nc) as tc, ExitStack() as ctx:
        singles = ctx.enter_context(tc.tile_pool(name="singles", bufs=1))
        stream = ctx.enter_context(tc.tile_pool(name="stream", bufs=2))
        extp = ctx.enter_context(tc.tile_pool(name="ext", bufs=2))
        smalls = ctx.enter_context(tc.tile_pool(name="smalls", bufs=1))
        psA = ctx.enter_context(tc.tile_pool(name="psA", bufs=2, space="PSUM"))
        psD = ctx.enter_context(tc.tile_pool(name="psD", bufs=2, space="PSUM"))

        # ---- constants (all APs are views into the packed blob) ----
        w1bf = singles.tile([D_IN, D_HID], bf16)
        nc.sync.dma_start(
            out=w1bf[:],
            in_=bass.AP(bv, off["W1"] // 2, [[D_HID, D_IN], [1, D_HID]]))
        wdbf = singles.tile([D_HID, 1], bf16)
        nc.sync.dma_start(
            out=wdbf[:], in_=bass.AP(bv, off["wd"] // 2, [[1, D_HID], [1, 1]]))
        sels = singles.tile([P, D_HID], f32)
        nc.sync.dma_start(
            out=sels[:],
            in_=bass.AP(fv, off["selmat"] // 4, [[D_HID, P], [1, D_HID]]))
        b1col = singles.tile([D_HID, 1], f32)
        nc.sync.dma_start(
            out=b1col[:], in_=bass.AP(fv, off["b1"] // 4, [[1, D_HID], [1, 1]]))
        bdsc = singles.tile([1, 1], f32)
        nc.sync.dma_start(
            out=bdsc[:], in_=bass.AP(fv, off["bd"] // 4, [[1, 1], [1, 1]]))
        dinvT = singles.tile([D_HID, NSH], bf16)
        nc.sync.dma_start(
            out=dinvT[:],
            in_=bass.AP(bv, off["dinv"] // 2, [[1, NSH]])
            .unsqueeze(0).to_broadcast([D_HID, NSH]))

        gidx = singles.tile([P, GC], i16)
        nc.sync.dma_start(out=gidx[:], in_=bass.AP(iv, off["gidx"] // 2,
                                                   [[GC, P], [1, GC]]))
        xidx = singles.tile([P, XC], i16)
        nc.sync.dma_start(out=xidx[:], in_=bass.AP(iv, off["xidx"] // 2,
                                                   [[XC, P], [1, XC]]))

        hs1T = singles.tile([D_HID, NSH], bf16)
        hs2T = singles.tile([D_HID, NSH], bf16)
        tableT = singles.tile([P, NSH], f32)

        # ---- phase A: hs1^T = W1^T @ (dinv * x)^T, straight from SBUF ----
        xsT = stream.tile([P, NSH], bf16, tag="big", name="xsT")
        nc.sync.dma_start(
            out=xsT[:], in_=bass.AP(bv, off["xsT"] // 2, [[NSH, P], [1, NSH]]))
        for blk in range(NBLK):
            cols = slice(blk * BLK, (blk + 1) * BLK)
            ps = psA.tile([D_HID, BLK], f32, space="PSUM", tag="agg")
            nc.tensor.matmul(out=ps[:], lhsT=w1bf[:], rhs=xsT[:, cols],
                             start=True, stop=True)
            nc.vector.tensor_copy(hs1T[:, cols], ps[:])

        # ---- shard -> feature-major full-graph table via bf16 AllGather ----
        def make_table(hsT, ag_in, table, tname):
            nc.sync.dma_start(out=ag_in.ap(), in_=hsT[:])
            nc.gpsimd.collective_compute(
                "AllGather", mybir.AluOpType.bypass, replica_groups=groups,
                ins=[ag_in.ap().opt()], outs=[table.ap().opt()],
            )
            tmp = stream.tile([P, NSH], bf16, tag="big", name=tname)
            nc.sync.dma_start(out=tmp[:], in_=table[:, :])
            nc.vector.tensor_copy(tableT[:], tmp[:])

        # ---- edge aggregation: hsT[:, d] += sum_banks(segment sums) ----
        def aggregate(hsT):
            for k in range(NCHUNK):
                msg = stream.tile([P, nidx], f32, tag="big", name=f"msg{k}")
                nc.gpsimd.ap_gather(
                    out_ap=msg[:], in_ap=tableT[:],
                    idxs_ap=gidx[:, k * (nidx // 16):(k + 1) * (nidx // 16)],
                    channels=P, num_elems=NSH, d=1, num_idxs=nidx,
                )
                nc.vector.memset(msg[:, 0:1], 0.0)
                scn = stream.tile([P, nidx], f32, tag="big", name=f"scn{k}")
                nc.vector.tensor_tensor_scan(
                    out=scn[:], data0=msg[:], data1=msg[:], initial=0.0,
                    op0=mybir.AluOpType.add, op1=mybir.AluOpType.bypass,
                )
                ex = extp.tile([P, nx], f32, tag="ex")
                nc.gpsimd.ap_gather(
                    out_ap=ex[:], in_ap=scn[:],
                    idxs_ap=xidx[:, k * (nx // 16):(k + 1) * (nx // 16)],
                    channels=P, num_elems=nidx, d=1, num_idxs=nx,
                )
                dif = extp.tile([P, DCH], f32, tag="dif")
                nc.vector.tensor_sub(dif[:], ex[:, 1:DCH + 1], ex[:, 0:DCH])
                for j in range(2):
                    ps = psA.tile([D_HID, BLK], f32, space="PSUM", tag="agg")
                    nc.tensor.matmul(
                        out=ps[:], lhsT=sels[:], rhs=dif[:, j * BLK:(j + 1) * BLK],
                        start=True, stop=True,
                    )
                    cols = slice(k * DCH + j * BLK, k * DCH + (j + 1) * BLK)
                    nc.vector.tensor_add(out=hsT[:, cols], in0=ps[:],
                                         in1=hsT[:, cols])

        make_table(hs1T, ag_in1, table1, "tb1")
        aggregate(hs1T)

        # ---- layer-1 epilogue (feature-major, in place) ----
        nc.vector.tensor_mul(out=hs1T[:], in0=hs1T[:], in1=dinvT[:])
        nc.scalar.activation(out=hs1T[:], in_=hs1T[:],
                             func=mybir.ActivationFunctionType.Relu,
                             bias=b1col[:])
        nc.vector.tensor_mul(out=hs2T[:], in0=hs1T[:], in1=dinvT[:])

        make_table(hs2T, ag_in2, table2, "tb2")
        aggregate(hs2T)

        # ---- layer-2 epilogue + closed-form 2-class log-softmax ----
        # t2 = dinv*(agg2+hs2); d = wd^T t2 + bd;
        # out0 = -softplus(d); out1 = d + out0
        nc.vector.tensor_mul(out=hs2T[:], in0=hs2T[:], in1=dinvT[:])

        for blk in range(NBLK):
            cols = slice(blk * BLK, (blk + 1) * BLK)
            ps = psD.tile([1, BLK], f32, space="PSUM", tag="d")
            nc.tensor.matmul(out=ps[:], lhsT=wdbf[:], rhs=hs2T[:, cols],
                             start=True, stop=True)
            ex_d = smalls.tile([1, BLK], f32, tag="e")
            nc.scalar.activation(out=ex_d[:], in_=ps[:],
                                 func=mybir.ActivationFunctionType.Exp,
                                 bias=bdsc[:])
            sp = smalls.tile([1, BLK], f32, tag="sp")
            nc.scalar.activation(out=sp[:], in_=ex_d[:],
                                 func=mybir.ActivationFunctionType.Ln,
                                 bias=1.0)
            o0 = smalls.tile([1, BLK], bf16, tag="o0")
            nc.vector.tensor_scalar_mul(o0[:], sp[:], -1.0)
            dsb = smalls.tile([1, BLK], f32, tag="dsb")
            nc.scalar.activation(out=dsb[:], in_=ps[:],
                                 func=mybir.ActivationFunctionType.Identity,
                                 bias=bdsc[:])
            o1 = smalls.tile([1, BLK], bf16, tag="o1")
            nc.vector.tensor_sub(o1[:], dsb[:], sp[:])
            nc.sync.dma_start(
                out=bass.AP(out_d, blk * BLK, [[NSH, 1], [1, BLK]]),
                in_=o0[:])
            nc.sync.dma_start(
                out=bass.AP(out_d, NSH + blk * BLK, [[NSH, 1], [1, BLK]]),
                in_=o1[:])

    nc.compile()
    return nc


def _build_noop():
    """Tiny program for calibrating the PJRT/axon transport overhead."""
    from contextlib import ExitStack

    import concourse.tile as tile
    from concourse import bacc, mybir

    f32 = mybir.dt.float32
    nc = bacc.Bacc(
        "TRN2", target_bir_lowering=False, debug=False,
        enable_asserts=False, num_devices=NCORES,
    )
    z_in = nc.dram_tensor("z_in", [P, P], f32, kind="ExternalInput")
    z_out = nc.dram_tensor("z_out", [P, P], f32, kind="ExternalOutput")
    with tile.TileContext(nc) as tc, ExitStack() as ctx:
        sb = ctx.enter_context(tc.tile_pool(name="sb", bufs=1))
        t = sb.tile([P, P], f32)
        nc.sync.dma_start(out=t[:], in_=z_in[:, :])
        nc.sync.dma_start(out=z_out[:, :], in_=t[:])
    nc.compile()
    return nc


_CACHE = {}


def _make_in_maps(inputs_np, gidx, xidx, dinv):
    import ml_dtypes

    nidx = gidx.shape[2] // NCHUNK * 16
    nx = xidx.shape[2] // NCHUNK * 16
    off, nb, GC, XC = _blob_layout(nidx, nx)

    x = np.asarray(inputs_np["x"], dtype=np.float32)
    dinv_pad = np.ones(NCORES * NSH, dtype=np.float32)
    dinv_pad[:N_NODES] = dinv
    xs_pad = np.zeros((NCORES * NSH, D_IN), dtype=np.float32)
    xs_pad[:N_NODES] = x * dinv[:, None]
    dinv_bf = dinv_pad.astype(ml_dtypes.bfloat16)
    selmat = np.tile(np.eye(D_HID, dtype=np.float32), (NBANK, 1))

    w1_bf = np.ascontiguousarray(
        np.asarray(inputs_np["W1"], dtype=np.float32)).astype(ml_dtypes.bfloat16)
    b1 = np.ascontiguousarray(np.asarray(inputs_np["b1"], dtype=np.float32))
    w2 = np.asarray(inputs_np["W2"], dtype=np.float32)
    b2 = np.asarray(inputs_np["b2"], dtype=np.float32)
    wd_bf = np.ascontiguousarray(w2[:, 1] - w2[:, 0]).astype(ml_dtypes.bfloat16)
    bd = np.array([b2[1] - b2[0]], dtype=np.float32)

    in_maps = []
    for c in range(NCORES):
        blob = np.zeros(nb, np.uint8)

        def put(o, arr):
            raw = np.ascontiguousarray(arr).view(np.uint8).reshape(-1)
            blob[o:o + raw.size] = raw

        xsT_c = np.ascontiguousarray(
            xs_pad[c * NSH:(c + 1) * NSH].T).astype(ml_dtypes.bfloat16)
        put(off["xsT"], xsT_c)
        put(off["gidx"], gidx[c])
        put(off["xidx"], xidx[c])
        put(off["dinv"], dinv_bf[c * NSH:(c + 1) * NSH])
        put(off["W1"], w1_bf)
        put(off["wd"], wd_bf)
        put(off["selmat"], selmat)
        put(off["b1"], b1)
        put(off["bd"], bd)
        in_maps.append({"blob": blob})
    return in_maps


_JAX_CACHE_SET = False


def _enable_jax_compile_cache():
    """Persistent XLA compilation cache: repeat kernel() calls skip the
    per-call backend recompile (fresh jit closures defeat the in-memory
    pjit cache)."""
    global _JAX_CACHE_SET
    if _JAX_CACHE_SET:
        return
    _JAX_CACHE_SET = True
    try:
        import jax

        jax.config.update("jax_compilation_cache_dir", "/tmp/jax_comp_cache")
        jax.config.update("jax_persistent_cache_min_entry_size_bytes", 0)
        jax.config.update("jax_persistent_cache_min_compile_time_secs", 0.0)
    except Exception:
        pass


def kernel(x, W1, b1, W2, b2, edge_index):
    from concourse.bass_utils import run_bass_kernel_spmd

    _enable_jax_compile_cache()
    inputs_np = {"x": x, "W1": W1, "b1": b1, "W2": W2, "b2": b2}
    edge_index = np.asarray(edge_index)

    gidx, xidx, dinv, nidx, nx = _host_prep(edge_index)

    key = (nidx, nx)
    if key not in _CACHE:
        _CACHE[key] = _build_program(nidx, nx)
    nc = _CACHE[key]

    in_maps = _make_in_maps(inputs_np, gidx, xidx, dinv)

    res = run_bass_kernel_spmd(nc, in_maps, core_ids=list(range(NCORES)))
    shards = [np.asarray(res.results[c]["out"], dtype=np.float32)
              for c in range(NCORES)]  # each [2, NSH] bf16 -> f32
    out = np.concatenate(shards, axis=1).T[:N_NODES]
    return np.ascontiguousarray(out.astype(np.float32))


# revision 26
# speedup vs baseline: 1.8982x; 1.5170x over previous
"""GCN 2-layer classifier on 8 TRN2 NeuronCores.

Strategy (dst-sharded graph parallel, feature-major end-to-end):
  - Nodes sharded 8 ways by id range (NSH=12544 logical rows per core, core 7
    zero-padded).  Each core receives ONLY its own inputs, packed into a
    single uint8 blob per core (one host->device transfer): (x*dinv)^T shard
    in bf16 (pre-scaled + pre-transposed on the host), int16 edge-index
    streams, bf16/f32 weights.
  - All node features on device live feature-major [16, NSH]: hs1 = W1^T @
    xsT is 28 PE matmuls straight out of SBUF (no transposes anywhere).
    The per-layer full-graph table is built by AllGathering the bf16
    [16, NSH] shard: the concatenation IS the table layout
    table_T[(bank, feat), node_in_bank] = [128, NSH], upconverted once to
    f32 in SBUF for the gather.
  - Edges sorted by dst on the host, bucketed per (core, src-bank,
    dst-range-chunk) into uniform-length int16 index streams (SPMD-identical
    structure, data differs per core).  Per chunk:
      * GPSIMD ap_gather pulls hs[src] along the free axis for all 8 banks in
        parallel (each Q7 core serves its bank's 16 feature partitions).
      * DVE tensor_tensor_scan computes a plain prefix sum over the
        dst-sorted message stream.
      * a second ap_gather extracts the prefix at per-dst segment boundaries;
        adjacent differences give per-(bank,dst) partial sums.
      * one PE matmul per 448 dsts contracts the partition axis against a
        block-identity selector, summing the 8 banks into feature-major
        [16, 448] partial sums, accumulated directly onto the hs shard
        (self-loop term pre-seeded).
  - Symmetric normalization folds into the tables: out = dinv*(agg+hs[d]) + b
    with hs = h*dinv, so there is no per-edge norm work.
  - Layer 2 aggregates 16-dim features first (A@h commutes with @W2); the
    final 2-class log-softmax uses the closed form out0 = -softplus(y1-y0),
    out1 = (y1-y0) + out0, needing only the single projected difference
    d = (W2[:,1]-W2[:,0])^T @ t2 + (b2[1]-b2[0]).
"""

import sys

import numpy as np

sys.path.insert(0, "/opt/trn_rl_repo")

N_NODES = 100000
N_EDGES = 3200000
D_IN, D_HID, D_OUT = 128, 16, 2
NCORES = 8
P = 128
NSH = 12544          # shard rows per core (98 * 128)
NCHUNK = 14          # dst-range chunks per core
DCH = NSH // NCHUNK  # 896 dsts per chunk
BLK = 448            # matmul free-dim block (2 per chunk, 28 per shard)
NBLK = NSH // BLK    # 28
NBANK = 8


def _host_prep(edge_index):
    """Sort edges by dst, bucket per (core, src-bank, dst-chunk), build
    uniform int16 gather/extraction index streams."""
    src = np.ascontiguousarray(edge_index[0]).astype(np.int64)
    dst = np.ascontiguousarray(edge_index[1]).astype(np.int64)

    deg = np.bincount(dst, minlength=N_NODES).astype(np.float64) + 1.0
    dinv = (1.0 / np.sqrt(deg)).astype(np.float32)

    order = np.argsort(dst, kind="stable")
    src_s = src[order]
    dst_s = dst[order]
    bank_s = src_s // NSH

    # cell id = ((core * NBANK) + bank) * NCHUNK + chunk, edges within a cell
    # stay dst-sorted under a stable sort by cell
    core_s = dst_s // NSH
    chunk_s = (dst_s % NSH) // DCH
    cell = (core_s * NBANK + bank_s) * NCHUNK + chunk_s
    cell_order = np.argsort(cell, kind="stable")
    src_c = src_s[cell_order]
    dst_c = dst_s[cell_order]
    cell_c = cell[cell_order]

    ncells = NCORES * NBANK * NCHUNK
    counts = np.bincount(cell_c, minlength=ncells)
    starts = np.zeros(ncells + 1, dtype=np.int64)
    np.cumsum(counts, out=starts[1:])

    # uniform padded stream length: slot 0 is a zero sentinel
    # round to multiples of 32 so every per-chunk int16 index slice starts
    # 4-byte aligned (GPSIMD reads indices in 32-bit words)
    nidx = int(counts.max()) + 1
    nidx = ((nidx + 31) // 32) * 32
    nx = DCH + 1
    nx = ((nx + 31) // 32) * 32

    gidx = np.zeros((NCORES, P, NCHUNK * (nidx // 16)), dtype=np.int16)
    xidx = np.zeros((NCORES, P, NCHUNK * (nx // 16)), dtype=np.int16)

    src_local = (src_c % NSH).astype(np.int32)
    rel_dst = (dst_c % NSH) % DCH

    for c in range(NCORES):
        for b in range(NBANK):
            rows = slice(b * 16, (b + 1) * 16)
            for k in range(NCHUNK):
                g = (c * NBANK + b) * NCHUNK + k
                a, e = starts[g], starts[g + 1]
                n = e - a
                # gather stream: [0] + bank-local src ids + pads(0)
                stream = np.zeros(nidx, dtype=np.int16)
                stream[1:1 + n] = src_local[a:e]
                gidx[c, rows, k * (nidx // 16):(k + 1) * (nidx // 16)] = (
                    stream.reshape(nidx // 16, 16).T
                )
                # extraction stream: prefix positions [0, cum(0), ..,
                # cum(DCH-1)] then pads repeating the last position
                cum = np.zeros(nx, dtype=np.int16)
                cnt = np.bincount(rel_dst[a:e], minlength=DCH)
                cum[1:DCH + 1] = np.cumsum(cnt)
                cum[DCH + 1:] = cum[DCH]
                xidx[c, rows, k * (nx // 16):(k + 1) * (nx // 16)] = (
                    cum.reshape(nx // 16, 16).T
                )

    return gidx, xidx, dinv, nidx, nx


def _blob_layout(nidx, nx, pad=0):
    """Byte offsets of each logical tensor inside the packed per-core blob.

    ``pad`` adds extra tail bytes; benchmark-only (it makes program variants
    have distinct XLA cache signatures)."""
    GC = NCHUNK * (nidx // 16)
    XC = NCHUNK * (nx // 16)
    off = {}
    nb = 0
    off["xsT"] = nb
    nb += D_IN * NSH              # int8, transposed x shard (scale is in W1)
    off["gidx"] = nb
    nb += P * GC * 2              # int16
    off["xidx"] = nb
    nb += P * XC * 2              # int16
    off["dinv"] = nb
    nb += NSH * 2                 # bf16
    off["W1"] = nb
    nb += D_IN * D_HID * 2        # bf16
    off["wd"] = nb
    nb += D_HID * 2               # bf16, W2[:,1]-W2[:,0]
    assert nb % 4 == 0
    off["selmat"] = nb
    nb += P * D_HID * 4           # f32
    off["b1"] = nb
    nb += D_HID * 4               # f32
    off["bd"] = nb
    nb += 4                       # f32, b2[1]-b2[0]
    nb = (nb + 511) // 512 * 512 + pad
    return off, nb, GC, XC


def _build_program(nidx, nx, variant="full", pad=0):
    from contextlib import ExitStack

    import concourse.bass as bass
    import concourse.tile as tile
    from concourse import bacc, mybir

    skip_agg = variant in ("noagg", "noagg_nocoll")
    skip_coll = variant in ("nocoll", "noagg_nocoll")
    skip_final = variant == "nofinal"

    f32 = mybir.dt.float32
    bf16 = mybir.dt.bfloat16
    i16 = mybir.dt.int16
    u8 = mybir.dt.uint8

    off, nb, GC, XC = _blob_layout(nidx, nx, pad=pad)

    nc = bacc.Bacc(
        "TRN2",
        target_bir_lowering=False,
        debug=False,
        enable_asserts=False,
        num_devices=NCORES,
    )

    # ---- kernel I/O: one packed input blob, one (transposed) bf16 output ----
    blob = nc.dram_tensor("blob", [nb], u8, kind="ExternalInput")
    out_d = nc.dram_tensor("out", [D_OUT, NSH], bf16, kind="ExternalOutput")

    i8 = mybir.dt.int8
    bv = blob.bitcast(bf16)
    iv = blob.bitcast(i16)
    fv = blob.bitcast(f32)
    i8v = blob.bitcast(i8)

    # internal DRAM: shard bounce + gathered tables (bf16 on the wire)
    ag_in1 = nc.dram_tensor("ag_in1", [D_HID, NSH], bf16)
    ag_in2 = nc.dram_tensor("ag_in2", [D_HID, NSH], bf16)
    table1 = nc.dram_tensor("table1", [P, NSH], bf16, addr_space="Shared")
    table2 = nc.dram_tensor("table2", [P, NSH], bf16, addr_space="Shared")

    groups = [list(range(NCORES))]

    if variant == "sink":
        # benchmark-only: same I/O, no compute
        with tile.TileContext(nc) as tc, ExitStack() as ctx:
            sb = ctx.enter_context(tc.tile_pool(name="sb", bufs=1))
            o = sb.tile([D_OUT, NSH], bf16)
            nc.vector.memset(o[:], 0.0)
            nc.sync.dma_start(out=out_d.ap(), in_=o[:])
        nc.compile()
        return nc

    with tile.TileContext(nc) as tc, ExitStack() as ctx:
        singles = ctx.enter_context(tc.tile_pool(name="singles", bufs=1))
        stream = ctx.enter_context(tc.tile_pool(name="stream", bufs=2))
        extp = ctx.enter_context(tc.tile_pool(name="ext", bufs=2))
        smalls = ctx.enter_context(tc.tile_pool(name="smalls", bufs=1))
        psA = ctx.enter_context(tc.tile_pool(name="psA", bufs=2, space="PSUM"))
        psD = ctx.enter_context(tc.tile_pool(name="psD", bufs=2, space="PSUM"))

        # ---- constants (all APs are views into the packed blob) ----
        w1bf = singles.tile([D_IN, D_HID], bf16)
        nc.sync.dma_start(
            out=w1bf[:],
            in_=bass.AP(bv, off["W1"] // 2, [[D_HID, D_IN], [1, D_HID]]))
        wdbf = singles.tile([D_HID, 1], bf16)
        nc.sync.dma_start(
            out=wdbf[:], in_=bass.AP(bv, off["wd"] // 2, [[1, D_HID], [1, 1]]))
        sels = singles.tile([P, D_HID], f32)
        nc.sync.dma_start(
            out=sels[:],
            in_=bass.AP(fv, off["selmat"] // 4, [[D_HID, P], [1, D_HID]]))
        b1col = singles.tile([D_HID, 1], f32)
        nc.sync.dma_start(
            out=b1col[:], in_=bass.AP(fv, off["b1"] // 4, [[1, D_HID], [1, 1]]))
        bdsc = singles.tile([1, 1], f32)
        nc.sync.dma_start(
            out=bdsc[:], in_=bass.AP(fv, off["bd"] // 4, [[1, 1], [1, 1]]))
        dinvT = singles.tile([D_HID, NSH], bf16)
        nc.sync.dma_start(
            out=dinvT[:],
            in_=bass.AP(bv, off["dinv"] // 2, [[1, NSH]])
            .unsqueeze(0).to_broadcast([D_HID, NSH]))

        gidx = singles.tile([P, GC], i16)
        nc.sync.dma_start(out=gidx[:], in_=bass.AP(iv, off["gidx"] // 2,
                                                   [[GC, P], [1, GC]]))
        xidx = singles.tile([P, XC], i16)
        nc.sync.dma_start(out=xidx[:], in_=bass.AP(iv, off["xidx"] // 2,
                                                   [[XC, P], [1, XC]]))

        hs1T = singles.tile([D_HID, NSH], bf16)
        hs2T = singles.tile([D_HID, NSH], bf16)
        tableT = singles.tile([P, NSH], f32)

        # ---- phase A: hs1^T = dinv * (W1'^T @ x^T), straight from SBUF ----
        # x arrives int8 with the quant scale folded into W1'; dinv scaling
        # fuses into the PSUM->SBUF copy against the resident dinvT row.
        xsT = stream.tile([P, NSH], i8, tag="big", name="xsT")
        nc.sync.dma_start(
            out=xsT[:], in_=bass.AP(i8v, off["xsT"], [[NSH, P], [1, NSH]]))
        for blk in range(NBLK):
            cols = slice(blk * BLK, (blk + 1) * BLK)
            xb = extp.tile([P, BLK], bf16, tag="xb")
            nc.vector.tensor_copy(xb[:], xsT[:, cols])
            ps = psA.tile([D_HID, BLK], f32, space="PSUM", tag="agg")
            nc.tensor.matmul(out=ps[:], lhsT=w1bf[:], rhs=xb[:],
                             start=True, stop=True)
            nc.vector.tensor_mul(out=hs1T[:, cols], in0=ps[:],
                                 in1=dinvT[:, cols])

        # ---- shard -> feature-major full-graph table via bf16 AllGather ----
        def make_table(hsT, ag_in, table, tname):
            if skip_coll:
                nc.vector.memset(tableT[:], 0.5)
                return
            nc.sync.dma_start(out=ag_in.ap(), in_=hsT[:])
            nc.gpsimd.collective_compute(
                "AllGather", mybir.AluOpType.bypass, replica_groups=groups,
                ins=[ag_in.ap().opt()], outs=[table.ap().opt()],
            )
            tmp = stream.tile([P, NSH], bf16, tag="big", name=tname)
            nc.sync.dma_start(out=tmp[:], in_=table[:, :])
            nc.vector.tensor_copy(tableT[:], tmp[:])

        # ---- edge aggregation: hsT[:, d] += sum_banks(segment sums) ----
        def aggregate(hsT):
            if skip_agg:
                return
            for k in range(NCHUNK):
                msg = stream.tile([P, nidx], f32, tag="big", name=f"msg{k}")
                nc.gpsimd.ap_gather(
                    out_ap=msg[:], in_ap=tableT[:],
                    idxs_ap=gidx[:, k * (nidx // 16):(k + 1) * (nidx // 16)],
                    channels=P, num_elems=NSH, d=1, num_idxs=nidx,
                )
                nc.vector.memset(msg[:, 0:1], 0.0)
                scn = stream.tile([P, nidx], f32, tag="big", name=f"scn{k}")
                nc.vector.tensor_tensor_scan(
                    out=scn[:], data0=msg[:], data1=msg[:], initial=0.0,
                    op0=mybir.AluOpType.add, op1=mybir.AluOpType.bypass,
                )
                ex = extp.tile([P, nx], f32, tag="ex")
                nc.gpsimd.ap_gather(
                    out_ap=ex[:], in_ap=scn[:],
                    idxs_ap=xidx[:, k * (nx // 16):(k + 1) * (nx // 16)],
                    channels=P, num_elems=nidx, d=1, num_idxs=nx,
                )
                dif = extp.tile([P, DCH], f32, tag="dif")
                nc.vector.tensor_sub(dif[:], ex[:, 1:DCH + 1], ex[:, 0:DCH])
                for j in range(DCH // BLK):
                    ps = psA.tile([D_HID, BLK], f32, space="PSUM", tag="agg")
                    nc.tensor.matmul(
                        out=ps[:], lhsT=sels[:], rhs=dif[:, j * BLK:(j + 1) * BLK],
                        start=True, stop=True,
                    )
                    cols = slice(k * DCH + j * BLK, k * DCH + (j + 1) * BLK)
                    nc.vector.tensor_add(out=hsT[:, cols], in0=ps[:],
                                         in1=hsT[:, cols])

        make_table(hs1T, ag_in1, table1, "tb1")
        aggregate(hs1T)

        # ---- layer-1 epilogue (feature-major, in place) ----
        nc.vector.tensor_mul(out=hs1T[:], in0=hs1T[:], in1=dinvT[:])
        nc.scalar.activation(out=hs1T[:], in_=hs1T[:],
                             func=mybir.ActivationFunctionType.Relu,
                             bias=b1col[:])
        nc.vector.tensor_mul(out=hs2T[:], in0=hs1T[:], in1=dinvT[:])

        make_table(hs2T, ag_in2, table2, "tb2")
        aggregate(hs2T)

        # ---- layer-2 epilogue + closed-form 2-class log-softmax ----
        # t2 = dinv*(agg2+hs2); d = wd^T t2 + bd;
        # out0 = -softplus(d); out1 = d + out0
        nc.vector.tensor_mul(out=hs2T[:], in0=hs2T[:], in1=dinvT[:])

        if skip_final:
            nc.sync.dma_start(out=out_d.ap(), in_=hs2T[0:D_OUT, :])

        for blk in range(0 if not skip_final else NBLK, NBLK):
            cols = slice(blk * BLK, (blk + 1) * BLK)
            ps = psD.tile([1, BLK], f32, space="PSUM", tag="d")
            nc.tensor.matmul(out=ps[:], lhsT=wdbf[:], rhs=hs2T[:, cols],
                             start=True, stop=True)
            ex_d = smalls.tile([1, BLK], f32, tag="e")
            nc.scalar.activation(out=ex_d[:], in_=ps[:],
                                 func=mybir.ActivationFunctionType.Exp,
                                 bias=bdsc[:])
            sp = smalls.tile([1, BLK], f32, tag="sp")
            nc.scalar.activation(out=sp[:], in_=ex_d[:],
                                 func=mybir.ActivationFunctionType.Ln,
                                 bias=1.0)
            o0 = smalls.tile([1, BLK], bf16, tag="o0")
            nc.vector.tensor_scalar_mul(o0[:], sp[:], -1.0)
            dsb = smalls.tile([1, BLK], f32, tag="dsb")
            nc.scalar.activation(out=dsb[:], in_=ps[:],
                                 func=mybir.ActivationFunctionType.Identity,
                                 bias=bdsc[:])
            o1 = smalls.tile([1, BLK], bf16, tag="o1")
            nc.vector.tensor_sub(o1[:], dsb[:], sp[:])
            nc.sync.dma_start(
                out=bass.AP(out_d, blk * BLK, [[NSH, 1], [1, BLK]]),
                in_=o0[:])
            nc.sync.dma_start(
                out=bass.AP(out_d, NSH + blk * BLK, [[NSH, 1], [1, BLK]]),
                in_=o1[:])

    nc.compile()
    return nc


def _build_noop():
    """Tiny program for calibrating the PJRT/axon transport overhead."""
    from contextlib import ExitStack

    import concourse.tile as tile
    from concourse import bacc, mybir

    f32 = mybir.dt.float32
    nc = bacc.Bacc(
        "TRN2", target_bir_lowering=False, debug=False,
        enable_asserts=False, num_devices=NCORES,
    )
    z_in = nc.dram_tensor("z_in", [P, P], f32, kind="ExternalInput")
    z_out = nc.dram_tensor("z_out", [P, P], f32, kind="ExternalOutput")
    with tile.TileContext(nc) as tc, ExitStack() as ctx:
        sb = ctx.enter_context(tc.tile_pool(name="sb", bufs=1))
        t = sb.tile([P, P], f32)
        nc.sync.dma_start(out=t[:], in_=z_in[:, :])
        nc.sync.dma_start(out=z_out[:, :], in_=t[:])
    nc.compile()
    return nc


_CACHE = {}


def _make_in_maps(inputs_np, gidx, xidx, dinv, pad=0):
    import ml_dtypes

    nidx = gidx.shape[2] // NCHUNK * 16
    nx = xidx.shape[2] // NCHUNK * 16
    off, nb, GC, XC = _blob_layout(nidx, nx, pad=pad)

    x = np.asarray(inputs_np["x"], dtype=np.float32)
    dinv_pad = np.ones(NCORES * NSH, dtype=np.float32)
    dinv_pad[:N_NODES] = dinv
    # int8-quantize x with a single global scale, folded into W1
    step = max(float(np.abs(x).max()), 1e-30) / 127.0
    xq_pad = np.zeros((NCORES * NSH, D_IN), dtype=np.int8)
    xq_pad[:N_NODES] = np.clip(np.rint(x / step), -127, 127).astype(np.int8)
    dinv_bf = dinv_pad.astype(ml_dtypes.bfloat16)
    selmat = np.tile(np.eye(D_HID, dtype=np.float32), (NBANK, 1))

    w1_bf = (np.asarray(inputs_np["W1"], dtype=np.float32) * step).astype(
        ml_dtypes.bfloat16)
    b1 = np.ascontiguousarray(np.asarray(inputs_np["b1"], dtype=np.float32))
    w2 = np.asarray(inputs_np["W2"], dtype=np.float32)
    b2 = np.asarray(inputs_np["b2"], dtype=np.float32)
    wd_bf = np.ascontiguousarray(w2[:, 1] - w2[:, 0]).astype(ml_dtypes.bfloat16)
    bd = np.array([b2[1] - b2[0]], dtype=np.float32)

    in_maps = []
    for c in range(NCORES):
        blob = np.zeros(nb, np.uint8)

        def put(o, arr):
            raw = np.ascontiguousarray(arr).view(np.uint8).reshape(-1)
            blob[o:o + raw.size] = raw

        xsT_c = np.ascontiguousarray(xq_pad[c * NSH:(c + 1) * NSH].T)
        put(off["xsT"], xsT_c)
        put(off["gidx"], gidx[c])
        put(off["xidx"], xidx[c])
        put(off["dinv"], dinv_bf[c * NSH:(c + 1) * NSH])
        put(off["W1"], w1_bf)
        put(off["wd"], wd_bf)
        put(off["selmat"], selmat)
        put(off["b1"], b1)
        put(off["bd"], bd)
        in_maps.append({"blob": blob})
    return in_maps


_JAX_CACHE_SET = False


def _enable_jax_compile_cache():
    """Persistent XLA compilation cache: repeat kernel() calls skip the
    per-call backend recompile (fresh jit closures defeat the in-memory
    pjit cache)."""
    global _JAX_CACHE_SET
    if _JAX_CACHE_SET:
        return
    _JAX_CACHE_SET = True
    try:
        import jax

        jax.config.update("jax_compilation_cache_dir", "/tmp/jax_comp_cache")
        jax.config.update("jax_persistent_cache_min_entry_size_bytes", 0)
        jax.config.update("jax_persistent_cache_min_compile_time_secs", 0.0)
    except Exception:
        pass


def kernel(x, W1, b1, W2, b2, edge_index):
    from concourse.bass_utils import run_bass_kernel_spmd

    _enable_jax_compile_cache()
    inputs_np = {"x": x, "W1": W1, "b1": b1, "W2": W2, "b2": b2}
    edge_index = np.asarray(edge_index)

    gidx, xidx, dinv, nidx, nx = _host_prep(edge_index)

    key = (nidx, nx)
    if key not in _CACHE:
        _CACHE[key] = _build_program(nidx, nx)
    nc = _CACHE[key]

    in_maps = _make_in_maps(inputs_np, gidx, xidx, dinv)

    res = run_bass_kernel_spmd(nc, in_maps, core_ids=list(range(NCORES)))
    shards = [np.asarray(res.results[c]["out"], dtype=np.float32)
              for c in range(NCORES)]  # each [2, NSH] bf16 -> f32
    out = np.concatenate(shards, axis=1).T[:N_NODES]
    return np.ascontiguousarray(out.astype(np.float32))
